# revision 35
# baseline (speedup 1.0000x reference)
"""Causal self-attention on 8 NeuronCores (Trainium2, Bass/Tile).

Sharding: core c handles batch b = c//2 and head-group hg = c%2
(8 of 16 heads = 512 of 1024 feature dims). W_qkv is split column-wise,
W_proj row-wise per head group; each core returns a partial [T, D]
projection output and the host sums the two partials per batch.

Per-core dataflow (all matmuls bf16; host pre-casts inputs):
  xT = embds[b].T              [1024, 2048]  (host-transposed, bf16)
  qT/kT = Wq/Wk.T @ x.T        [512, 2048]   (head-dim major, bf16)
  v     = x @ Wv               [2048, 512]   (natural, + ones col per head)
  sT[j,i] = kT.T @ qT          per head-PAIR: two K=64 matmuls row-tiled
            onto disjoint halves of the PE array (partitions 0-63 even
            head, 64-127 odd head) run concurrently -> 2x score rate
  PT = exp(SCALE * sT) (*mask on diagonal strips)   one ACT op / pair
  UT[e,i], denom[i] = [v|1].T @ PT per head         (ones col -> denom)
  affinT = UT * (1/denom)      broadcast via K=8 matmul with E matrix
  partial = affinT.T @ Wp      accumulated over e-chunks, DMA'd out
"""

import sys

for _p in ("/opt/trn_rl_repo",):
    if _p not in sys.path:
        sys.path.append(_p)

import ml_dtypes
import numpy as np

import concourse.bass as bass
import concourse.tile as tile
from concourse import bacc, mybir
from concourse.bass_utils import run_bass_kernel_spmd

F32 = mybir.dt.float32
BF16 = mybir.dt.bfloat16
F8 = mybir.dt.float8e4
DR = mybir.MatmulPerfMode.DoubleRow
EXP = mybir.ActivationFunctionType.Exp
COPY = mybir.ActivationFunctionType.Copy

B, T, D = 4, 2048, 1024
H, Dh = 16, 64
SCALE = float(D) ** -0.5
NCORES = 8
DL = 512          # local (per-core) feature width = 8 heads * 64
HL = 8            # local heads
NDC = D // 128    # 8 d-chunks
NEC = DL // 128   # 4 e-chunks (head pairs)
NTB = T // 512    # 4 t-blocks of 512
NTC = T // 128    # 16 t-chunks of 128
VPAIR = 192       # v_sb per-pair block: [v_even(64) | one | junk(63) | v_odd(64)]
VROW = NEC * VPAIR  # 640 cols per v_sb tile


def _build():
    nc = bacc.Bacc("TRN2", target_bir_lowering=False, debug=False,
                   num_devices=NCORES)

    xT = nc.declare_dram_parameter("xT", [D, T], BF16, isOutput=False)
    xT8 = nc.declare_dram_parameter("xT8", [D, T], F8, isOutput=False)
    wq = nc.declare_dram_parameter("wq", [D, DL], F8, isOutput=False)
    wk = nc.declare_dram_parameter("wk", [D, DL], F8, isOutput=False)
    wv = nc.declare_dram_parameter("wv", [D, DL], BF16, isOutput=False)
    wp = nc.declare_dram_parameter("wp", [DL, D], BF16, isOutput=False)
    mask = nc.declare_dram_parameter("mask", [128, 128], BF16, isOutput=False)
    emat = nc.declare_dram_parameter("emat", [HL, DL], BF16, isOutput=False)
    outA = nc.declare_dram_parameter("outA", [T, D], F32, isOutput=True)

    with tile.TileContext(nc) as tc:
        _emit(nc, tc, xT, xT8, wq, wk, wv, wp, mask, emat, outA)
    nc.compile()
    return nc


def _emit(nc, tc, xT, xT8, wq, wk, wv, wp, mask, emat, outA):
    from contextlib import ExitStack

    ctx = ExitStack()
    with ctx:
        wqk_pool = ctx.enter_context(tc.tile_pool(name="wqk", bufs=16))
        wv_pool = ctx.enter_context(tc.tile_pool(name="wv", bufs=NDC))
        qk_pool = ctx.enter_context(tc.tile_pool(name="qk", bufs=4))
        vsb_pool = ctx.enter_context(tc.tile_pool(name="vsb", bufs=NTC))
        ut_pool = ctx.enter_context(tc.tile_pool(name="ut", bufs=NEC))
        dn_pool = ctx.enter_context(tc.tile_pool(name="dn", bufs=1))
        dns_pool = ctx.enter_context(tc.tile_pool(name="dns", bufs=2))
        pt_pool = ctx.enter_context(tc.tile_pool(name="pt", bufs=4))
        cst_pool = ctx.enter_context(tc.tile_pool(name="cst", bufs=1))
        ps_pool = ctx.enter_context(tc.tile_pool(name="ps", bufs=1, space="PSUM"))

        # constants (mask staged twice so one strided DVE multiply covers
        # both heads' diagonal blocks)
        mk_sb = cst_pool.tile([128, 256], BF16, tag="mk")
        nc.sync.dma_start(mk_sb[:, 0:128], mask[:])
        nc.sync.dma_start(mk_sb[:, 128:256], mask[:])
        em_sb = cst_pool.tile([HL, DL], BF16, tag="em")
        nc.sync.dma_start(em_sb[:], emat[:])

        # persistent tiles
        ut_sb = [ut_pool.tile([128, T], BF16, tag="ut", name=f"ut{i}")
                 for i in range(NEC)]
        dn_sb = dn_pool.tile([HL, T], F32, tag="dn")
        rd_sb = dn_pool.tile([HL, T], F32, tag="rd")
        rdb_sb = dn_pool.tile([HL, T], BF16, tag="rdb")
        v_sb = [vsb_pool.tile([128, VROW], BF16, tag="vsb", name=f"vsb{i}")
                for i in range(NTC)]
        # garbage rows of dn would hit reciprocal before they are written;
        # keep them finite so 0*inf NaNs can't leak out of the R matmul
        nc.gpsimd.memset(dn_sb[:], 1.0)

        def ps_tile(tag, bufs, cols=512):
            return ps_pool.tile([128, cols], F32, tag=tag, name=f"ps_{tag}",
                                bufs=bufs)

        with tc.tile_pool(name="xt", bufs=NDC) as xt_pool:
            xt_sb = []
            xt8_sb = []
            qk_chunks = {}

            def qk_weights(ec):
                """DMA the fp8 q/k weight tiles for chunk `ec`, allocate its
                q_ec/k_ec output tiles.  q/k are stored head-dim-major:
                even head of the pair on partitions 0-63, odd on 64-127
                (feeds the row-tiled score matmul pairs)."""
                w_ts = []
                for w_src in (wq, wk):
                    w_t = []
                    for p in range(NDC // 2):
                        t = wqk_pool.tile([128, 256], F8, tag="wqk",
                                          name="wqkt")
                        for o in range(2):
                            nc.sync.dma_start(
                                t[:, o * 128:(o + 1) * 128],
                                w_src[(2 * p + o) * 128:(2 * p + o + 1) * 128,
                                      ec * 128:(ec + 1) * 128])
                        w_t.append(t)
                    w_ts.append(w_t)
                q_ec = qk_pool.tile([128, T], BF16, tag="qk", name="q_ec")
                k_ec = qk_pool.tile([128, T], BF16, tag="qk", name="k_ec")
                qk_chunks[ec] = (q_ec, k_ec)
                return ((q_ec, w_ts[0]), (k_ec, w_ts[1]))

            def qk_part(st, pairs):
                """Generator emitting the fp8 DoubleRow projection matmuls
                for the given (dst_idx, tb) pairs; yields between small PE
                steps so it can be dripped into the attention loop as
                filler work that keeps the PE dense (HAM stays warm)."""
                for di, tb in pairs:
                    dst, w_t = st[di]
                    ps_q = ps_tile("qkps", 2)
                    for p in range(NDC // 2):
                        w3 = w_t[p][:].rearrange("k (o m) -> k o m", o=2)
                        x3 = xt8_sb[p][:].rearrange(
                            "k (o t) -> k o t", o=2)[
                            :, :, tb * 512:(tb + 1) * 512]
                        nc.tensor.matmul(
                            ps_q[:], w3, x3,
                            start=(p == 0), stop=(p == NDC // 2 - 1),
                            perf_mode=DR)
                        yield
                    nc.vector.tensor_copy(
                        dst[:, tb * 512:(tb + 1) * 512], ps_q[:])
                    yield

            def qk_filler(ec):
                st = qk_weights(ec)
                yield from qk_part(
                    st, [(di, tb) for di in range(2) for tb in range(NTB)])

            def normalize_tb(ec, tb):
                """affinT = UT * 1/denom for chunk ec, t-block tb (the
                recip -> cast -> broadcast -> scale chain pipelines)."""
                sl = slice(tb * 512, (tb + 1) * 512)
                nc.vector.reciprocal_approx_fast(rd_sb[:, sl], dn_sb[:, sl])
                nc.gpsimd.tensor_copy(rdb_sb[:, sl], rd_sb[:, sl])
                ps_r = ps_tile("qkps", 2)
                nc.tensor.matmul(
                    ps_r[:], em_sb[:, ec * 128:(ec + 1) * 128],
                    rdb_sb[:, sl], start=True, stop=True)
                nc.vector.tensor_mul(
                    ut_sb[ec][:, sl], ut_sb[ec][:, sl], ps_r[:])

            # ------------- phase A0: DMAs + first v blocks + qk(0) tb0 -------------
            wv_sb = []
            for dc in range(NDC):
                t = wv_pool.tile([128, DL], BF16, tag="wv", name=f"wv{dc}")
                nc.sync.dma_start(t[:], wv[dc * 128:(dc + 1) * 128, :])
                wv_sb.append(t)
                t = xt_pool.tile([128, T], BF16, tag="xt", name=f"xt{dc}")
                # small first slice of every chunk lands first so the
                # first v accumulations can start ~1us in; the bulk
                # follows in progressively larger (2KB-line) transfers
                nc.sync.dma_start(t[:, 0:T // 8],
                                  xT[dc * 128:(dc + 1) * 128, 0:T // 8])
                xt_sb.append(t)
            for dc in range(NDC):
                nc.sync.dma_start(xt_sb[dc][:, T // 8:T // 4],
                                  xT[dc * 128:(dc + 1) * 128,
                                     T // 8:T // 4])
            # fp8 copy of x for the q/k projections, d-chunk pairs
            # packed side by side per tile (feeds DoubleRow matmuls);
            # after xT[0:512] (early v) but before the xT bulk
            for p in range(NDC // 2):
                t = xt_pool.tile([128, 2 * T], F8, tag="xt8",
                                 name=f"xt8_{p}", bufs=NDC // 2)
                for o in range(2):
                    nc.sync.dma_start(
                        t[:, o * T:(o + 1) * T],
                        xT8[(2 * p + o) * 128:(2 * p + o + 1) * 128, :])
                xt8_sb.append(t)
            for lo, hi in ((T // 4, T // 2), (T // 2, T)):
                for dc in range(NDC):
                    nc.sync.dma_start(xt_sb[dc][:, lo:hi],
                                      xT[dc * 128:(dc + 1) * 128, lo:hi])

            def v_unit(tcn):
                """One v = x @ Wv token-chunk, as a drippable generator."""
                ps_v = ps_tile("qkps", 2)
                for dc in range(NDC):
                    nc.tensor.matmul(
                        ps_v[:], xt_sb[dc][:, tcn * 128:(tcn + 1) * 128],
                        wv_sb[dc][:], start=(dc == 0),
                        stop=(dc == NDC - 1))
                    if dc % 3 == 2:
                        yield
                dst = v_sb[tcn][:].rearrange("p (e c) -> p e c", c=VPAIR)
                src = ps_v[:].rearrange("p (e c) -> p e c", c=128)
                nc.vector.tensor_copy(dst[:, :, 0:64], src[:, :, 0:64])
                nc.vector.tensor_copy(dst[:, :, 128:192], src[:, :, 64:128])
                nc.gpsimd.memset(dst[:, :, 64:65], 1.0)
                nc.gpsimd.memset(dst[:, :, 65:128], 0.0)
                yield

            # prologue: v blocks 0-3 and q/k t-block 0 (everything the
            # first attention i-block needs); the rest of v and qk(0) is
            # dripped just-in-time into attention(0) below, overlapping
            # this projection work with the exp-bound attention steps
            st0 = qk_weights(0)
            for tcn in range(4):
                for _ in v_unit(tcn):
                    pass
            for _ in qk_part(st0, [(0, 0), (1, 0)]):
                pass

            # ------------- per e-chunk: attention + dripped filler work -------------
            def attention_chunk(ec, drip, on_flush=None):
                """Head pair 2ec/2ec+1.  Per (i-block, j-chunk) step: two
                K=64 score matmuls row-tiled onto disjoint halves of the PE
                array, one exp over both heads' scores, one PV matmul per
                head.  Software-pipelined one step deep: step t's PV pair is
                emitted after step t+1's scores+exp, so the PE queue never
                heads into an exp-wait while score work exists."""
                q_ec, k_ec = qk_chunks.pop(ec)
                steps = [(ib, jt) for ib in range(NTB)
                         for jt in range(4 * ib + 4)]
                ut_ps = [None, None]
                pend = []

                def flush():
                    if not pend:
                        return
                    ib, jt, cl, pt_t = pend.pop(0)
                    if jt == 0:
                        ut_ps[0] = ps_tile("utps", 2)
                        ut_ps[1] = ps_tile("utps", 2)
                    utE, utO = ut_ps
                    # PV: [v|1].T @ PT -> UT rows + denom row
                    vt = v_sb[jt][:].rearrange(
                        "p (e c) -> p e c", c=VPAIR)[:, ec, :]
                    last = (jt == 4 * ib + 3)
                    nc.tensor.matmul(
                        utE[0:65, cl:512], vt[:, 0:65], pt_t[:, cl:512],
                        start=(jt == 0), stop=last, skip_group_check=True)
                    nc.tensor.matmul(
                        utO[0:128, cl:512], vt[:, 64:192],
                        pt_t[:, 512 + cl:1024],
                        start=(jt == 0), stop=last, skip_group_check=True)
                    if not last:
                        return
                    for par, ut_d in ((0, utE), (1, utO)):
                        h = 2 * ec + par
                        if par == 0:
                            usrc, dsrc, r = (ut_d[0:64, :],
                                             ut_d[64:65, :], 64)
                            udst = ut_sb[ec][0:64, ib * 512:(ib + 1) * 512]
                        else:
                            # [one|junk63|v_odd]: denom row 0, v 64..127
                            usrc, dsrc, r = (ut_d[64:128, :],
                                             ut_d[0:1, :], 0)
                            udst = ut_sb[ec][64:128, ib * 512:(ib + 1) * 512]
                        with tc.high_priority():
                            nc.vector.tensor_copy(udst, usrc)
                            # denom: same-partition copy + DMA repack
                            stg = dns_pool.tile([128, 512], F32, tag="dns",
                                                name="dnstg")
                            nc.vector.tensor_copy(stg[r:r + 1, :], dsrc)
                        nc.sync.dma_start(
                            dn_sb[h:h + 1, ib * 512:(ib + 1) * 512],
                            stg[r:r + 1, :])
                    if on_flush is not None:
                        on_flush(ib)

                for idx, (ib, jt) in enumerate(steps):
                    diag = (jt // 4 == ib)
                    cl = 128 * (jt - 4 * ib) if diag else 0
                    isl = slice(ib * 512 + cl, (ib + 1) * 512)
                    s_ps = ps_pool.tile([128, 1024], F32, tag="stps",
                                        name="ps_stps", bufs=2)
                    kj = k_ec[:, jt * 128:(jt + 1) * 128]
                    nc.tensor.matmul(
                        s_ps[:, cl:512], kj[0:64, :], q_ec[0:64, isl],
                        start=True, stop=True)
                    nc.tensor.matmul(
                        s_ps[:, 512 + cl:1024], kj[64:128, :],
                        q_ec[64:128, isl], start=True, stop=True)
                    pt_t = pt_pool.tile([128, 1024], BF16, tag="pt")
                    if cl == 0:
                        nc.scalar.activation(pt_t[:], s_ps[:], EXP,
                                             scale=SCALE)
                    else:
                        pt_v = pt_t[:].rearrange("p (b c) -> p b c", c=512)
                        sp_v = s_ps[:].rearrange("p (b c) -> p b c", c=512)
                        nc.scalar.activation(pt_v[:, :, cl:512],
                                             sp_v[:, :, cl:512], EXP,
                                             scale=SCALE)
                    if diag:
                        pt_v = pt_t[:].rearrange("p (b c) -> p b c", c=512)
                        nc.vector.tensor_mul(
                            pt_v[:, :, cl:cl + 128],
                            pt_v[:, :, cl:cl + 128],
                            mk_sb[:].rearrange("p (b c) -> p b c", c=128))
                    drip(idx)
                    if len(pend) >= 2:
                        flush()
                    pend.append((ib, jt, cl, pt_t))
                flush()
                flush()

            from itertools import chain as _chain

            # deadline-ordered drip for attention(0): remaining qk(0)
            # t-blocks and v blocks, each emitted before its first
            # consumer step, then qk(1)
            master0 = _chain(
                qk_part(st0, [(0, 1)]), qk_part(st0, [(1, 1)]),
                v_unit(4), v_unit(5), v_unit(6), v_unit(7),
                qk_part(st0, [(0, 2)]), v_unit(8), v_unit(9),
                qk_part(st0, [(1, 2)]), v_unit(10), v_unit(11),
                qk_part(st0, [(0, 3)]), v_unit(12), v_unit(13),
                qk_part(st0, [(1, 3)]), v_unit(14), v_unit(15),
                qk_filler(1))

            def drip0(slot):
                for _ in range(3):
                    next(master0, None)

            attention_chunk(0, drip0)
            for _ in master0:
                pass

            for ec in range(1, NEC - 1):
                filler = qk_filler(ec + 1)

                def drip(slot, ec=ec, filler=filler):
                    if 0 <= slot < 4:
                        normalize_tb(ec - 1, slot)
                    next(filler, None)

                attention_chunk(ec, drip)
                for _ in filler:   # drain remaining qk(ec+1) work
                    pass
        # xt pool released here (before the last attention chunk)

        with tc.tile_pool(name="wp", bufs=NEC) as wp_pool, \
             tc.tile_pool(name="stage", bufs=6) as stage_pool:
            wp_sb = []
            for ecn in range(NEC):
                t = wp_pool.tile([128, D], BF16, tag="wp", name=f"wpt{ecn}")
                nc.sync.dma_start(t[:], wp[ecn * 128:(ecn + 1) * 128, :])
                wp_sb.append(t)

            def proj_unit(ecs, out_t, tcn, ob, tag="qkps", act_copy=False):
                """One projection unit accumulating a subset of e-chunks
                into its own partial output (summed on the host)."""
                ps_p = ps_tile(tag, 2)
                for i, ecn in enumerate(ecs):
                    nc.tensor.matmul(
                        ps_p[:],
                        ut_sb[ecn][:, tcn * 128:(tcn + 1) * 128],
                        wp_sb[ecn][:, ob * 512:(ob + 1) * 512],
                        start=(i == 0), stop=(i == len(ecs) - 1))
                st = stage_pool.tile([128, 512], F32, tag="st", name="stg")
                if act_copy:
                    nc.scalar.activation(st[:], ps_p[:], COPY)
                else:
                    nc.vector.tensor_copy(st[:], ps_p[:])
                nc.sync.dma_start(
                    out_t[tcn * 128:(tcn + 1) * 128,
                          ob * 512:(ob + 1) * 512], st[:])

            # last attention chunk: one full projection pass (all 4 chunks),
            # i-block tb of chunk 3 unlocked (normalized) per flushed ib so
            # only the ib=3 slice remains for the tail.
            ECS = tuple(range(NEC))
            projU = {ib: [(tcn, ob) for tcn in range(4 * ib, 4 * ib + 4)
                          for ob in range(2)] for ib in range(NTB)}

            def drip3(slot):
                if slot < 4:
                    normalize_tb(NEC - 2, slot)
                elif slot >= 14:
                    ib = min((slot - 14) // 8, 2)
                    units = projU[ib]
                    # hold back 3 ungated ib2 units to cover the tail's
                    # denominator DMA -> recip -> broadcast latency
                    if units and (ib < 2 or len(units) > 3):
                        tcn, ob = units.pop(0)
                        proj_unit(ECS, outA, tcn, ob,
                                  act_copy=(slot % 2 == 1))

            def on_flush3(ib):
                normalize_tb(NEC - 1, ib)

            attention_chunk(NEC - 1, drip3, on_flush=on_flush3)
            tail = [u for ib in range(NTB) for u in projU[ib]]
            for i, u in enumerate(tail):
                proj_unit(ECS, outA, *u,
                          tag=("qkps" if i % 2 == 0 else "utps"),
                          act_copy=(i % 2 == 0))



_NC_CACHE = None


def _get_nc():
    global _NC_CACHE
    if _NC_CACHE is None:
        _NC_CACHE = _build()
    return _NC_CACHE


def make_in_maps(embds, W_qkv, W_proj):
    embds = np.asarray(embds, dtype=np.float32)
    W_qkv = np.asarray(W_qkv, dtype=np.float32)
    W_proj = np.asarray(W_proj, dtype=np.float32)
    bf16 = ml_dtypes.bfloat16

    f8 = ml_dtypes.float8_e4m3

    mask_np = np.triu(np.ones((128, 128))).astype(bf16)
    emat_np = np.kron(np.eye(HL), np.ones((1, Dh))).astype(bf16)

    in_maps = []
    for c in range(NCORES):
        b, hg = c // 2, c % 2
        sl = slice(hg * DL, (hg + 1) * DL)
        xt = np.ascontiguousarray(embds[b].T)
        in_maps.append({
            "xT": xt.astype(bf16),
            "xT8": xt.astype(f8),
            "wk": np.ascontiguousarray(W_qkv[:, 0 * D:1 * D][:, sl]).astype(f8),
            "wq": np.ascontiguousarray(W_qkv[:, 1 * D:2 * D][:, sl]).astype(f8),
            "wv": np.ascontiguousarray(W_qkv[:, 2 * D:3 * D][:, sl]).astype(bf16),
            "wp": np.ascontiguousarray(W_proj[sl, :]).astype(bf16),
            "mask": mask_np,
            "emat": emat_np,
        })
    return in_maps


def gather_out(outs, b_proj):
    b_proj = np.asarray(b_proj, dtype=np.float32)
    full = np.empty((B, T, D), dtype=np.float32)
    for b in range(B):
        full[b] = outs[2 * b] + outs[2 * b + 1] + b_proj[None, :]
    return full


def kernel(embds, W_qkv, W_proj, b_proj):
    in_maps = make_in_maps(embds, W_qkv, W_proj)
    nc = _get_nc()
    res = run_bass_kernel_spmd(nc, in_maps, list(range(NCORES)))
    return gather_out([np.asarray(r["outA"], dtype=np.float32)
                       for r in res.results], b_proj)


# revision 37
# speedup vs baseline: 1.0071x; 1.0071x over previous
"""Causal self-attention on 8 NeuronCores (Trainium2, Bass/Tile).

Sharding: core c handles batch b = c//2 and head-group hg = c%2
(8 of 16 heads = 512 of 1024 feature dims). W_qkv is split column-wise,
W_proj row-wise per head group; each core returns a partial [T, D]
projection output and the host sums the two partials per batch.

Per-core dataflow (all matmuls bf16; host pre-casts inputs):
  xT = embds[b].T              [1024, 2048]  (host-transposed, bf16)
  qT/kT = Wq/Wk.T @ x.T        [512, 2048]   (head-dim major, bf16)
  v     = x @ Wv               [2048, 512]   (natural, + ones col per head)
  sT[j,i] = kT.T @ qT          per head-PAIR: two K=64 matmuls row-tiled
            onto disjoint halves of the PE array (partitions 0-63 even
            head, 64-127 odd head) run concurrently -> 2x score rate
  PT = exp(SCALE * sT) (*mask on diagonal strips)   one ACT op / pair
  UT[e,i], denom[i] = [v|1].T @ PT per head         (ones col -> denom)
  affinT = UT * (1/denom)      broadcast via K=8 matmul with E matrix
  partial = affinT.T @ Wp      accumulated over e-chunks, DMA'd out
"""

import sys

for _p in ("/opt/trn_rl_repo",):
    if _p not in sys.path:
        sys.path.append(_p)

import ml_dtypes
import numpy as np

import concourse.bass as bass
import concourse.tile as tile
from concourse import bacc, mybir
from concourse.bass_utils import run_bass_kernel_spmd

F32 = mybir.dt.float32
BF16 = mybir.dt.bfloat16
F8 = mybir.dt.float8e4
DR = mybir.MatmulPerfMode.DoubleRow
EXP = mybir.ActivationFunctionType.Exp
COPY = mybir.ActivationFunctionType.Copy

B, T, D = 4, 2048, 1024
H, Dh = 16, 64
SCALE = float(D) ** -0.5
NCORES = 8
DL = 512          # local (per-core) feature width = 8 heads * 64
HL = 8            # local heads
NDC = D // 128    # 8 d-chunks
NEC = DL // 128   # 4 e-chunks (head pairs)
NTB = T // 512    # 4 t-blocks of 512
NTC = T // 128    # 16 t-chunks of 128
VPAIR = 192       # v_sb per-pair block: [v_even(64) | one | junk(63) | v_odd(64)]
VROW = NEC * VPAIR  # 640 cols per v_sb tile


def _build():
    nc = bacc.Bacc("TRN2", target_bir_lowering=False, debug=False,
                   num_devices=NCORES)

    xT = nc.declare_dram_parameter("xT", [D, T], BF16, isOutput=False)
    xT8 = nc.declare_dram_parameter("xT8", [D, T], F8, isOutput=False)
    wq = nc.declare_dram_parameter("wq", [D, DL], F8, isOutput=False)
    wk = nc.declare_dram_parameter("wk", [D, DL], F8, isOutput=False)
    wv = nc.declare_dram_parameter("wv", [D, DL], BF16, isOutput=False)
    wp = nc.declare_dram_parameter("wp", [DL, D], BF16, isOutput=False)
    mask = nc.declare_dram_parameter("mask", [128, 128], BF16, isOutput=False)
    emat = nc.declare_dram_parameter("emat", [HL, DL], BF16, isOutput=False)
    outA = nc.declare_dram_parameter("outA", [T, D], F32, isOutput=True)

    with tile.TileContext(nc) as tc:
        _emit(nc, tc, xT, xT8, wq, wk, wv, wp, mask, emat, outA)
    nc.compile()
    return nc


def _emit(nc, tc, xT, xT8, wq, wk, wv, wp, mask, emat, outA):
    from contextlib import ExitStack

    ctx = ExitStack()
    with ctx:
        wqk_pool = ctx.enter_context(tc.tile_pool(name="wqk", bufs=16))
        wv_pool = ctx.enter_context(tc.tile_pool(name="wv", bufs=NDC))
        qk_pool = ctx.enter_context(tc.tile_pool(name="qk", bufs=4))
        vsb_pool = ctx.enter_context(tc.tile_pool(name="vsb", bufs=NTC))
        ut_pool = ctx.enter_context(tc.tile_pool(name="ut", bufs=NEC))
        dn_pool = ctx.enter_context(tc.tile_pool(name="dn", bufs=1))
        dns_pool = ctx.enter_context(tc.tile_pool(name="dns", bufs=2))
        pt_pool = ctx.enter_context(tc.tile_pool(name="pt", bufs=4))
        cst_pool = ctx.enter_context(tc.tile_pool(name="cst", bufs=1))
        ps_pool = ctx.enter_context(tc.tile_pool(name="ps", bufs=1, space="PSUM"))

        # constants (mask staged twice so one strided DVE multiply covers
        # both heads' diagonal blocks)
        mk_sb = cst_pool.tile([128, 256], BF16, tag="mk")
        nc.sync.dma_start(mk_sb[:, 0:128], mask[:])
        nc.sync.dma_start(mk_sb[:, 128:256], mask[:])
        em_sb = cst_pool.tile([HL, DL], BF16, tag="em")
        nc.sync.dma_start(em_sb[:], emat[:])

        # persistent tiles
        ut_sb = [ut_pool.tile([128, T], BF16, tag="ut", name=f"ut{i}")
                 for i in range(NEC)]
        dn_sb = dn_pool.tile([HL, T], F32, tag="dn")
        rd_sb = dn_pool.tile([HL, T], F32, tag="rd")
        rdb_sb = dn_pool.tile([HL, T], BF16, tag="rdb")
        v_sb = [vsb_pool.tile([128, VROW], BF16, tag="vsb", name=f"vsb{i}")
                for i in range(NTC)]
        # garbage rows of dn would hit reciprocal before they are written;
        # keep them finite so 0*inf NaNs can't leak out of the R matmul
        nc.gpsimd.memset(dn_sb[:], 1.0)

        def ps_tile(tag, bufs, cols=512):
            return ps_pool.tile([128, cols], F32, tag=tag, name=f"ps_{tag}",
                                bufs=bufs)

        with tc.tile_pool(name="xt", bufs=NDC) as xt_pool:
            xt_sb = []
            xt8_sb = []
            qk_chunks = {}

            def qk_weights(ec):
                """DMA the fp8 q/k weight tiles for chunk `ec`, allocate its
                q_ec/k_ec output tiles.  q/k are stored head-dim-major:
                even head of the pair on partitions 0-63, odd on 64-127
                (feeds the row-tiled score matmul pairs)."""
                w_ts = []
                for w_src in (wq, wk):
                    w_t = []
                    for p in range(NDC // 2):
                        t = wqk_pool.tile([128, 256], F8, tag="wqk",
                                          name="wqkt")
                        for o in range(2):
                            nc.sync.dma_start(
                                t[:, o * 128:(o + 1) * 128],
                                w_src[(2 * p + o) * 128:(2 * p + o + 1) * 128,
                                      ec * 128:(ec + 1) * 128])
                        w_t.append(t)
                    w_ts.append(w_t)
                q_ec = qk_pool.tile([128, T], BF16, tag="qk", name="q_ec")
                k_ec = qk_pool.tile([128, T], BF16, tag="qk", name="k_ec")
                qk_chunks[ec] = (q_ec, k_ec)
                return ((q_ec, w_ts[0]), (k_ec, w_ts[1]))

            def qk_part(st, pairs):
                """Generator emitting the fp8 DoubleRow projection matmuls
                for the given (dst_idx, tb) pairs; yields between small PE
                steps so it can be dripped into the attention loop as
                filler work that keeps the PE dense (HAM stays warm)."""
                for di, tb in pairs:
                    dst, w_t = st[di]
                    ps_q = ps_tile("qkps", 2)
                    for p in range(NDC // 2):
                        w3 = w_t[p][:].rearrange("k (o m) -> k o m", o=2)
                        x3 = xt8_sb[p][:].rearrange(
                            "k (o t) -> k o t", o=2)[
                            :, :, tb * 512:(tb + 1) * 512]
                        nc.tensor.matmul(
                            ps_q[:], w3, x3,
                            start=(p == 0), stop=(p == NDC // 2 - 1),
                            perf_mode=DR)
                        yield
                    nc.vector.tensor_copy(
                        dst[:, tb * 512:(tb + 1) * 512], ps_q[:])
                    yield

            def qk_filler(ec):
                st = qk_weights(ec)
                yield from qk_part(
                    st, [(di, tb) for di in range(2) for tb in range(NTB)])

            def normalize_tb(ec, tb):
                """affinT = UT * 1/denom for chunk ec, t-block tb (the
                recip -> cast -> broadcast -> scale chain pipelines)."""
                sl = slice(tb * 512, (tb + 1) * 512)
                nc.vector.reciprocal_approx_fast(rd_sb[:, sl], dn_sb[:, sl])
                nc.gpsimd.tensor_copy(rdb_sb[:, sl], rd_sb[:, sl])
                ps_r = ps_tile("qkps", 2)
                nc.tensor.matmul(
                    ps_r[:], em_sb[:, ec * 128:(ec + 1) * 128],
                    rdb_sb[:, sl], start=True, stop=True)
                nc.vector.tensor_mul(
                    ut_sb[ec][:, sl], ut_sb[ec][:, sl], ps_r[:])

            # ------------- phase A0: DMAs + first v blocks + qk(0) tb0 -------------
            wv_sb = []
            for dc in range(NDC):
                t = wv_pool.tile([128, DL], BF16, tag="wv", name=f"wv{dc}")
                nc.sync.dma_start(t[:], wv[dc * 128:(dc + 1) * 128, :])
                wv_sb.append(t)
                t = xt_pool.tile([128, T], BF16, tag="xt", name=f"xt{dc}")
                # small first slice of every chunk lands first so the
                # first v accumulations can start ~1us in; the bulk
                # follows in progressively larger (2KB-line) transfers
                nc.sync.dma_start(t[:, 0:T // 8],
                                  xT[dc * 128:(dc + 1) * 128, 0:T // 8])
                xt_sb.append(t)
            for dc in range(NDC):
                nc.sync.dma_start(xt_sb[dc][:, T // 8:T // 4],
                                  xT[dc * 128:(dc + 1) * 128,
                                     T // 8:T // 4])
            # fp8 copy of x for the q/k projections, d-chunk pairs
            # packed side by side per tile (feeds DoubleRow matmuls);
            # after xT[0:512] (early v) but before the xT bulk
            for p in range(NDC // 2):
                t = xt_pool.tile([128, 2 * T], F8, tag="xt8",
                                 name=f"xt8_{p}", bufs=NDC // 2)
                for o in range(2):
                    nc.sync.dma_start(
                        t[:, o * T:(o + 1) * T],
                        xT8[(2 * p + o) * 128:(2 * p + o + 1) * 128, :])
                xt8_sb.append(t)
            for lo, hi in ((T // 4, T // 2), (T // 2, T)):
                for dc in range(NDC):
                    nc.sync.dma_start(xt_sb[dc][:, lo:hi],
                                      xT[dc * 128:(dc + 1) * 128, lo:hi])

            # v = x @ Wv with qk(0) dripped in as filler
            filler0 = qk_filler(0)
            for tcn in range(NTC):
                ps_v = ps_tile("utps", 2)
                for dc in range(NDC):
                    nc.tensor.matmul(
                        ps_v[:], xt_sb[dc][:, tcn * 128:(tcn + 1) * 128],
                        wv_sb[dc][:], start=(dc == 0),
                        stop=(dc == NDC - 1))
                dst = v_sb[tcn][:].rearrange("p (e c) -> p e c", c=VPAIR)
                src = ps_v[:].rearrange("p (e c) -> p e c", c=128)
                nc.vector.tensor_copy(dst[:, :, 0:64], src[:, :, 0:64])
                nc.vector.tensor_copy(dst[:, :, 128:192], src[:, :, 64:128])
                nc.gpsimd.memset(dst[:, :, 64:65], 1.0)
                nc.gpsimd.memset(dst[:, :, 65:128], 0.0)
                next(filler0, None)
            for _ in filler0:
                pass

            # ------------- per e-chunk: attention + dripped filler work -------------
            def attention_chunk(ec, drip, on_flush=None):
                """Head pair 2ec/2ec+1.  Per (i-block, j-chunk) step: two
                K=64 score matmuls row-tiled onto disjoint halves of the PE
                array, one exp over both heads' scores, one PV matmul per
                head.  Software-pipelined one step deep: step t's PV pair is
                emitted after step t+1's scores+exp, so the PE queue never
                heads into an exp-wait while score work exists."""
                q_ec, k_ec = qk_chunks.pop(ec)
                steps = [(ib, jt) for ib in range(NTB)
                         for jt in range(4 * ib + 4)]
                ut_ps = [None, None]
                pend = []

                def flush():
                    if not pend:
                        return
                    ib, jt, cl, pt_t = pend.pop(0)
                    if jt == 0:
                        ut_ps[0] = ps_tile("utps", 2)
                        ut_ps[1] = ps_tile("utps", 2)
                    utE, utO = ut_ps
                    # PV: [v|1].T @ PT -> UT rows + denom row
                    vt = v_sb[jt][:].rearrange(
                        "p (e c) -> p e c", c=VPAIR)[:, ec, :]
                    last = (jt == 4 * ib + 3)
                    nc.tensor.matmul(
                        utE[0:65, cl:512], vt[:, 0:65], pt_t[:, cl:512],
                        start=(jt == 0), stop=last, skip_group_check=True)
                    nc.tensor.matmul(
                        utO[0:128, cl:512], vt[:, 64:192],
                        pt_t[:, 512 + cl:1024],
                        start=(jt == 0), stop=last, skip_group_check=True)
                    if not last:
                        return
                    for par, ut_d in ((0, utE), (1, utO)):
                        h = 2 * ec + par
                        if par == 0:
                            usrc, dsrc, r = (ut_d[0:64, :],
                                             ut_d[64:65, :], 64)
                            udst = ut_sb[ec][0:64, ib * 512:(ib + 1) * 512]
                        else:
                            # [one|junk63|v_odd]: denom row 0, v 64..127
                            usrc, dsrc, r = (ut_d[64:128, :],
                                             ut_d[0:1, :], 0)
                            udst = ut_sb[ec][64:128, ib * 512:(ib + 1) * 512]
                        with tc.high_priority():
                            nc.vector.tensor_copy(udst, usrc)
                            # denom: same-partition copy + DMA repack
                            stg = dns_pool.tile([128, 512], F32, tag="dns",
                                                name="dnstg")
                            nc.vector.tensor_copy(stg[r:r + 1, :], dsrc)
                        nc.sync.dma_start(
                            dn_sb[h:h + 1, ib * 512:(ib + 1) * 512],
                            stg[r:r + 1, :])
                    if on_flush is not None:
                        on_flush(ib)

                for idx, (ib, jt) in enumerate(steps):
                    diag = (jt // 4 == ib)
                    cl = 128 * (jt - 4 * ib) if diag else 0
                    isl = slice(ib * 512 + cl, (ib + 1) * 512)
                    s_ps = ps_pool.tile([128, 1024], F32, tag="stps",
                                        name="ps_stps", bufs=2)
                    kj = k_ec[:, jt * 128:(jt + 1) * 128]
                    nc.tensor.matmul(
                        s_ps[:, cl:512], kj[0:64, :], q_ec[0:64, isl],
                        start=True, stop=True)
                    nc.tensor.matmul(
                        s_ps[:, 512 + cl:1024], kj[64:128, :],
                        q_ec[64:128, isl], start=True, stop=True)
                    pt_t = pt_pool.tile([128, 1024], BF16, tag="pt")
                    if cl == 0:
                        nc.scalar.activation(pt_t[:], s_ps[:], EXP,
                                             scale=SCALE)
                    else:
                        pt_v = pt_t[:].rearrange("p (b c) -> p b c", c=512)
                        sp_v = s_ps[:].rearrange("p (b c) -> p b c", c=512)
                        nc.scalar.activation(pt_v[:, :, cl:512],
                                             sp_v[:, :, cl:512], EXP,
                                             scale=SCALE)
                    if diag:
                        pt_v = pt_t[:].rearrange("p (b c) -> p b c", c=512)
                        nc.vector.tensor_mul(
                            pt_v[:, :, cl:cl + 128],
                            pt_v[:, :, cl:cl + 128],
                            mk_sb[:].rearrange("p (b c) -> p b c", c=128))
                    drip(idx)
                    if len(pend) >= 2:
                        flush()
                    pend.append((ib, jt, cl, pt_t))
                flush()
                flush()

            for ec in range(NEC - 1):
                filler = qk_filler(ec + 1)

                def drip(slot, ec=ec, filler=filler):
                    if ec > 0 and 0 <= slot < 4:
                        normalize_tb(ec - 1, slot)
                    next(filler, None)

                attention_chunk(ec, drip)
                for _ in filler:   # drain remaining qk(ec+1) work
                    pass
        # xt pool released here (before the last attention chunk)

        with tc.tile_pool(name="wp", bufs=NEC) as wp_pool, \
             tc.tile_pool(name="stage", bufs=6) as stage_pool:
            wp_sb = []
            for ecn in range(NEC):
                t = wp_pool.tile([128, D], BF16, tag="wp", name=f"wpt{ecn}")
                nc.sync.dma_start(t[:], wp[ecn * 128:(ecn + 1) * 128, :])
                wp_sb.append(t)

            def proj_unit(ecs, out_t, tcn, ob, tag="qkps", act_copy=False):
                """One projection unit accumulating a subset of e-chunks
                into its own partial output (summed on the host)."""
                ps_p = ps_tile(tag, 2)
                for i, ecn in enumerate(ecs):
                    nc.tensor.matmul(
                        ps_p[:],
                        ut_sb[ecn][:, tcn * 128:(tcn + 1) * 128],
                        wp_sb[ecn][:, ob * 512:(ob + 1) * 512],
                        start=(i == 0), stop=(i == len(ecs) - 1))
                st = stage_pool.tile([128, 512], F32, tag="st", name="stg")
                if act_copy:
                    nc.scalar.activation(st[:], ps_p[:], COPY)
                else:
                    nc.vector.tensor_copy(st[:], ps_p[:])
                nc.sync.dma_start(
                    out_t[tcn * 128:(tcn + 1) * 128,
                          ob * 512:(ob + 1) * 512], st[:])

            # last attention chunk: one full projection pass (all 4 chunks),
            # i-block tb of chunk 3 unlocked (normalized) per flushed ib so
            # only the ib=3 slice remains for the tail.
            ECS = tuple(range(NEC))
            projU = {ib: [(tcn, ob) for tcn in range(4 * ib, 4 * ib + 4)
                          for ob in range(2)] for ib in range(NTB)}

            def drip3(slot):
                if slot < 4:
                    normalize_tb(NEC - 2, slot)
                elif slot >= 14:
                    ib = min((slot - 14) // 8, 2)
                    units = projU[ib]
                    # hold back 3 ungated ib2 units to cover the tail's
                    # denominator DMA -> recip -> broadcast latency
                    if units and (ib < 2 or len(units) > 3):
                        tcn, ob = units.pop(0)
                        proj_unit(ECS, outA, tcn, ob,
                                  act_copy=(slot % 2 == 1))

            def on_flush3(ib):
                normalize_tb(NEC - 1, ib)

            attention_chunk(NEC - 1, drip3, on_flush=on_flush3)
            tail = [u for ib in range(NTB) for u in projU[ib]]
            for i, u in enumerate(tail):
                proj_unit(ECS, outA, *u,
                          tag=("qkps" if i % 2 == 0 else "utps"),
                          act_copy=(i % 2 == 0))



_NC_CACHE = None


def _get_nc():
    global _NC_CACHE
    if _NC_CACHE is None:
        _NC_CACHE = _build()
    return _NC_CACHE


def make_in_maps(embds, W_qkv, W_proj):
    embds = np.asarray(embds, dtype=np.float32)
    W_qkv = np.asarray(W_qkv, dtype=np.float32)
    W_proj = np.asarray(W_proj, dtype=np.float32)
    bf16 = ml_dtypes.bfloat16

    f8 = ml_dtypes.float8_e4m3

    mask_np = np.triu(np.ones((128, 128))).astype(bf16)
    emat_np = np.kron(np.eye(HL), np.ones((1, Dh))).astype(bf16)

    in_maps = []
    for c in range(NCORES):
        b, hg = c // 2, c % 2
        sl = slice(hg * DL, (hg + 1) * DL)
        xt = np.ascontiguousarray(embds[b].T)
        in_maps.append({
            "xT": xt.astype(bf16),
            "xT8": xt.astype(f8),
            "wk": np.ascontiguousarray(W_qkv[:, 0 * D:1 * D][:, sl]).astype(f8),
            "wq": np.ascontiguousarray(W_qkv[:, 1 * D:2 * D][:, sl]).astype(f8),
            "wv": np.ascontiguousarray(W_qkv[:, 2 * D:3 * D][:, sl]).astype(bf16),
            "wp": np.ascontiguousarray(W_proj[sl, :]).astype(bf16),
            "mask": mask_np,
            "emat": emat_np,
        })
    return in_maps


def gather_out(outs, b_proj):
    b_proj = np.asarray(b_proj, dtype=np.float32)
    full = np.empty((B, T, D), dtype=np.float32)
    for b in range(B):
        full[b] = outs[2 * b] + outs[2 * b + 1] + b_proj[None, :]
    return full


def kernel(embds, W_qkv, W_proj, b_proj):
    in_maps = make_in_maps(embds, W_qkv, W_proj)
    nc = _get_nc()
    res = run_bass_kernel_spmd(nc, in_maps, list(range(NCORES)))
    return gather_out([np.asarray(r["outA"], dtype=np.float32)
                       for r in res.results], b_proj)


# revision 41
# speedup vs baseline: 1.0255x; 1.0182x over previous
"""Causal self-attention on 8 NeuronCores (Trainium2, Bass/Tile).

Sharding: core c handles batch b = c//2 and head-group hg = c%2
(8 of 16 heads = 512 of 1024 feature dims). W_qkv is split column-wise,
W_proj row-wise per head group; each core returns a partial [T, D]
projection output and the host sums the two partials per batch.

Per-core dataflow (all matmuls bf16; host pre-casts inputs):
  xT = embds[b].T              [1024, 2048]  (host-transposed, bf16)
  qT/kT = Wq/Wk.T @ x.T        [512, 2048]   (head-dim major, bf16)
  v     = x @ Wv               [2048, 512]   (natural, + ones col per head)
  sT[j,i] = kT.T @ qT          per head-PAIR: two K=64 matmuls row-tiled
            onto disjoint halves of the PE array (partitions 0-63 even
            head, 64-127 odd head) run concurrently -> 2x score rate
  PT = exp(SCALE * sT) (*mask on diagonal strips)   one ACT op / pair
  UT[e,i], denom[i] = [v|1].T @ PT per head         (ones col -> denom)
  affinT = UT * (1/denom)      broadcast via K=8 matmul with E matrix
  partial = affinT.T @ Wp      accumulated over e-chunks, DMA'd out
"""

import sys

for _p in ("/opt/trn_rl_repo",):
    if _p not in sys.path:
        sys.path.append(_p)

import ml_dtypes
import numpy as np

import concourse.bass as bass
import concourse.tile as tile
from concourse import bacc, mybir
from concourse.bass_utils import run_bass_kernel_spmd

F32 = mybir.dt.float32
BF16 = mybir.dt.bfloat16
F8 = mybir.dt.float8e4
DR = mybir.MatmulPerfMode.DoubleRow
EXP = mybir.ActivationFunctionType.Exp
COPY = mybir.ActivationFunctionType.Copy

B, T, D = 4, 2048, 1024
H, Dh = 16, 64
SCALE = float(D) ** -0.5
NCORES = 8
DL = 512          # local (per-core) feature width = 8 heads * 64
HL = 8            # local heads
NDC = D // 128    # 8 d-chunks
NEC = DL // 128   # 4 e-chunks (head pairs)
NTB = T // 512    # 4 t-blocks of 512
NTC = T // 128    # 16 t-chunks of 128
VPAIR = 192       # v_sb per-pair block: [v_even(64) | one | junk(63) | v_odd(64)]
VROW = NEC * VPAIR  # 640 cols per v_sb tile


def _build():
    nc = bacc.Bacc("TRN2", target_bir_lowering=False, debug=False,
                   num_devices=NCORES)

    xT = nc.declare_dram_parameter("xT", [D, T], BF16, isOutput=False)
    xT8 = nc.declare_dram_parameter("xT8", [D, T], F8, isOutput=False)
    wq = nc.declare_dram_parameter("wq", [D, DL], F8, isOutput=False)
    wk = nc.declare_dram_parameter("wk", [D, DL], F8, isOutput=False)
    wv = nc.declare_dram_parameter("wv", [D, DL], BF16, isOutput=False)
    wp = nc.declare_dram_parameter("wp", [DL, D], BF16, isOutput=False)
    mask = nc.declare_dram_parameter("mask", [128, 128], BF16, isOutput=False)
    emat = nc.declare_dram_parameter("emat", [HL, DL], BF16, isOutput=False)
    outA = nc.declare_dram_parameter("outA", [T, D], F32, isOutput=True)

    with tile.TileContext(nc) as tc:
        _emit(nc, tc, xT, xT8, wq, wk, wv, wp, mask, emat, outA)
    nc.compile()
    return nc


def _emit(nc, tc, xT, xT8, wq, wk, wv, wp, mask, emat, outA):
    from contextlib import ExitStack

    ctx = ExitStack()
    with ctx:
        wqk_pool = ctx.enter_context(tc.tile_pool(name="wqk", bufs=16))
        wv_pool = ctx.enter_context(tc.tile_pool(name="wv", bufs=NDC))
        qk_pool = ctx.enter_context(tc.tile_pool(name="qk", bufs=4))
        vsb_pool = ctx.enter_context(tc.tile_pool(name="vsb", bufs=NTC))
        ut_pool = ctx.enter_context(tc.tile_pool(name="ut", bufs=NEC))
        dn_pool = ctx.enter_context(tc.tile_pool(name="dn", bufs=1))
        dns_pool = ctx.enter_context(tc.tile_pool(name="dns", bufs=2))
        pt_pool = ctx.enter_context(tc.tile_pool(name="pt", bufs=4))
        cst_pool = ctx.enter_context(tc.tile_pool(name="cst", bufs=1))
        ps_pool = ctx.enter_context(tc.tile_pool(name="ps", bufs=1, space="PSUM"))

        # constants (mask staged twice so one strided DVE multiply covers
        # both heads' diagonal blocks)
        mk_sb = cst_pool.tile([128, 256], BF16, tag="mk")
        nc.sync.dma_start(mk_sb[:, 0:128], mask[:])
        nc.sync.dma_start(mk_sb[:, 128:256], mask[:])
        em_sb = cst_pool.tile([HL, DL], BF16, tag="em")
        nc.sync.dma_start(em_sb[:], emat[:])

        # persistent tiles
        ut_sb = [ut_pool.tile([128, T], BF16, tag="ut", name=f"ut{i}")
                 for i in range(NEC)]
        dn_sb = dn_pool.tile([HL, T], F32, tag="dn")
        rd_sb = dn_pool.tile([HL, T], F32, tag="rd")
        rdb_sb = dn_pool.tile([HL, T], BF16, tag="rdb")
        v_sb = [vsb_pool.tile([128, VROW], BF16, tag="vsb", name=f"vsb{i}")
                for i in range(NTC)]
        # garbage rows of dn would hit reciprocal before they are written;
        # keep them finite so 0*inf NaNs can't leak out of the R matmul
        nc.gpsimd.memset(dn_sb[:], 1.0)

        def ps_tile(tag, bufs, cols=512):
            return ps_pool.tile([128, cols], F32, tag=tag, name=f"ps_{tag}",
                                bufs=bufs)

        with tc.tile_pool(name="xt", bufs=NDC) as xt_pool:
            xt_sb = []
            xt8_sb = []
            qk_chunks = {}

            def qk_weights(ec):
                """DMA the fp8 q/k weight tiles for chunk `ec`, allocate its
                q_ec/k_ec output tiles.  q/k are stored head-dim-major:
                even head of the pair on partitions 0-63, odd on 64-127
                (feeds the row-tiled score matmul pairs)."""
                w_ts = []
                for w_src in (wq, wk):
                    w_t = []
                    for p in range(NDC // 2):
                        t = wqk_pool.tile([128, 256], F8, tag="wqk",
                                          name="wqkt")
                        for o in range(2):
                            nc.sync.dma_start(
                                t[:, o * 128:(o + 1) * 128],
                                w_src[(2 * p + o) * 128:(2 * p + o + 1) * 128,
                                      ec * 128:(ec + 1) * 128])
                        w_t.append(t)
                    w_ts.append(w_t)
                q_ec = qk_pool.tile([128, T], BF16, tag="qk", name="q_ec")
                k_ec = qk_pool.tile([128, T], BF16, tag="qk", name="k_ec")
                qk_chunks[ec] = (q_ec, k_ec)
                return ((q_ec, w_ts[0]), (k_ec, w_ts[1]))

            def qk_part(st, pairs):
                """Generator emitting the fp8 DoubleRow projection matmuls
                for the given (dst_idx, tb) pairs; yields between small PE
                steps so it can be dripped into the attention loop as
                filler work that keeps the PE dense (HAM stays warm)."""
                for di, tb in pairs:
                    dst, w_t = st[di]
                    ps_q = ps_tile("qkps", 2)
                    for p in range(NDC // 2):
                        w3 = w_t[p][:].rearrange("k (o m) -> k o m", o=2)
                        x3 = xt8_sb[p][:].rearrange(
                            "k (o t) -> k o t", o=2)[
                            :, :, tb * 512:(tb + 1) * 512]
                        nc.tensor.matmul(
                            ps_q[:], w3, x3,
                            start=(p == 0), stop=(p == NDC // 2 - 1),
                            perf_mode=DR)
                        yield
                    nc.vector.tensor_copy(
                        dst[:, tb * 512:(tb + 1) * 512], ps_q[:])
                    yield

            def qk_filler(ec):
                st = qk_weights(ec)
                yield from qk_part(
                    st, [(di, tb) for di in range(2) for tb in range(NTB)])

            def normalize_tb(ec, tb):
                """affinT = UT * 1/denom for chunk ec, t-block tb (the
                recip -> cast -> broadcast -> scale chain pipelines)."""
                sl = slice(tb * 512, (tb + 1) * 512)
                nc.vector.reciprocal_approx_fast(rd_sb[:, sl], dn_sb[:, sl])
                nc.gpsimd.tensor_copy(rdb_sb[:, sl], rd_sb[:, sl])
                ps_r = ps_tile("qkps", 2)
                nc.tensor.matmul(
                    ps_r[:], em_sb[:, ec * 128:(ec + 1) * 128],
                    rdb_sb[:, sl], start=True, stop=True)
                nc.vector.tensor_mul(
                    ut_sb[ec][:, sl], ut_sb[ec][:, sl], ps_r[:])

            # ------------- phase A0: DMAs + first v blocks + qk(0) tb0 -------------
            wv_sb = []
            for dc in range(NDC):
                t = wv_pool.tile([128, DL], BF16, tag="wv", name=f"wv{dc}")
                nc.sync.dma_start(t[:], wv[dc * 128:(dc + 1) * 128, :])
                wv_sb.append(t)
                t = xt_pool.tile([128, T], BF16, tag="xt", name=f"xt{dc}")
                # small first slice of every chunk lands first so the
                # first v accumulations can start ~1us in; the bulk
                # follows in progressively larger (2KB-line) transfers
                nc.sync.dma_start(t[:, 0:T // 8],
                                  xT[dc * 128:(dc + 1) * 128, 0:T // 8])
                xt_sb.append(t)
            for dc in range(NDC):
                nc.sync.dma_start(xt_sb[dc][:, T // 8:T // 4],
                                  xT[dc * 128:(dc + 1) * 128,
                                     T // 8:T // 4])
            # fp8 copy of x for the q/k projections, d-chunk pairs
            # packed side by side per tile (feeds DoubleRow matmuls);
            # after xT[0:512] (early v) but before the xT bulk
            for p in range(NDC // 2):
                t = xt_pool.tile([128, 2 * T], F8, tag="xt8",
                                 name=f"xt8_{p}", bufs=NDC // 2)
                for o in range(2):
                    nc.sync.dma_start(
                        t[:, o * T:(o + 1) * T],
                        xT8[(2 * p + o) * 128:(2 * p + o + 1) * 128, :])
                xt8_sb.append(t)
            for lo, hi in ((T // 4, T // 2), (T // 2, T)):
                for dc in range(NDC):
                    nc.sync.dma_start(xt_sb[dc][:, lo:hi],
                                      xT[dc * 128:(dc + 1) * 128, lo:hi])

            # v = x @ Wv with qk(0) dripped in as filler
            filler0 = qk_filler(0)
            for tcn in range(NTC):
                ps_v = ps_tile("utps", 2)
                for dc in range(NDC):
                    nc.tensor.matmul(
                        ps_v[:], xt_sb[dc][:, tcn * 128:(tcn + 1) * 128],
                        wv_sb[dc][:], start=(dc == 0),
                        stop=(dc == NDC - 1))
                dst = v_sb[tcn][:].rearrange("p (e c) -> p e c", c=VPAIR)
                src = ps_v[:].rearrange("p (e c) -> p e c", c=128)
                nc.vector.tensor_copy(dst[:, :, 0:64], src[:, :, 0:64])
                nc.vector.tensor_copy(dst[:, :, 128:192], src[:, :, 64:128])
                nc.gpsimd.memset(dst[:, :, 64:65], 1.0)
                nc.gpsimd.memset(dst[:, :, 65:128], 0.0)
                # qk(0)'s first matmuls wait on the xt8 DMA; dripping them
                # early would stall the in-order PE queue behind that wait,
                # so only start once the fp8 x copy has landed (~tcn 6)
                if tcn >= 6:
                    next(filler0, None)
                    next(filler0, None)
            for _ in filler0:
                pass

            # ------------- per e-chunk: attention + dripped filler work -------------
            def attention_chunk(ec, drip, on_flush=None):
                """Head pair 2ec/2ec+1.  Per (i-block, j-chunk) step: two
                K=64 score matmuls row-tiled onto disjoint halves of the PE
                array, one exp over both heads' scores, one PV matmul per
                head.  Software-pipelined one step deep: step t's PV pair is
                emitted after step t+1's scores+exp, so the PE queue never
                heads into an exp-wait while score work exists."""
                q_ec, k_ec = qk_chunks.pop(ec)
                steps = [(ib, jt) for ib in range(NTB)
                         for jt in range(4 * ib + 4)]
                ut_ps = [None, None]
                pend = []

                def flush():
                    if not pend:
                        return
                    ib, jt, cl, pt_t = pend.pop(0)
                    if jt == 0:
                        ut_ps[0] = ps_tile("utps", 2)
                        ut_ps[1] = ps_tile("utps", 2)
                    utE, utO = ut_ps
                    # PV: [v|1].T @ PT -> UT rows + denom row
                    vt = v_sb[jt][:].rearrange(
                        "p (e c) -> p e c", c=VPAIR)[:, ec, :]
                    last = (jt == 4 * ib + 3)
                    nc.tensor.matmul(
                        utE[0:65, cl:512], vt[:, 0:65], pt_t[:, cl:512],
                        start=(jt == 0), stop=last, skip_group_check=True)
                    nc.tensor.matmul(
                        utO[0:128, cl:512], vt[:, 64:192],
                        pt_t[:, 512 + cl:1024],
                        start=(jt == 0), stop=last, skip_group_check=True)
                    if not last:
                        return
                    for par, ut_d in ((0, utE), (1, utO)):
                        h = 2 * ec + par
                        if par == 0:
                            usrc, dsrc, r = (ut_d[0:64, :],
                                             ut_d[64:65, :], 64)
                            udst = ut_sb[ec][0:64, ib * 512:(ib + 1) * 512]
                        else:
                            # [one|junk63|v_odd]: denom row 0, v 64..127
                            usrc, dsrc, r = (ut_d[64:128, :],
                                             ut_d[0:1, :], 0)
                            udst = ut_sb[ec][64:128, ib * 512:(ib + 1) * 512]
                        with tc.high_priority():
                            nc.vector.tensor_copy(udst, usrc)
                            # denom: same-partition copy + DMA repack
                            stg = dns_pool.tile([128, 512], F32, tag="dns",
                                                name="dnstg")
                            nc.vector.tensor_copy(stg[r:r + 1, :], dsrc)
                        nc.sync.dma_start(
                            dn_sb[h:h + 1, ib * 512:(ib + 1) * 512],
                            stg[r:r + 1, :])
                    if on_flush is not None:
                        on_flush(ib)

                for idx, (ib, jt) in enumerate(steps):
                    diag = (jt // 4 == ib)
                    cl = 128 * (jt - 4 * ib) if diag else 0
                    isl = slice(ib * 512 + cl, (ib + 1) * 512)
                    s_ps = ps_pool.tile([128, 1024], F32, tag="stps",
                                        name="ps_stps", bufs=2)
                    kj = k_ec[:, jt * 128:(jt + 1) * 128]
                    nc.tensor.matmul(
                        s_ps[:, cl:512], kj[0:64, :], q_ec[0:64, isl],
                        start=True, stop=True)
                    nc.tensor.matmul(
                        s_ps[:, 512 + cl:1024], kj[64:128, :],
                        q_ec[64:128, isl], start=True, stop=True)
                    pt_t = pt_pool.tile([128, 1024], BF16, tag="pt")
                    if cl == 0:
                        nc.scalar.activation(pt_t[:], s_ps[:], EXP,
                                             scale=SCALE)
                    else:
                        pt_v = pt_t[:].rearrange("p (b c) -> p b c", c=512)
                        sp_v = s_ps[:].rearrange("p (b c) -> p b c", c=512)
                        nc.scalar.activation(pt_v[:, :, cl:512],
                                             sp_v[:, :, cl:512], EXP,
                                             scale=SCALE)
                    if diag:
                        pt_v = pt_t[:].rearrange("p (b c) -> p b c", c=512)
                        nc.vector.tensor_mul(
                            pt_v[:, :, cl:cl + 128],
                            pt_v[:, :, cl:cl + 128],
                            mk_sb[:].rearrange("p (b c) -> p b c", c=128))
                    drip(idx)
                    if len(pend) >= 2:
                        flush()
                    pend.append((ib, jt, cl, pt_t))
                flush()
                flush()

            for ec in range(NEC - 1):
                filler = qk_filler(ec + 1)

                def drip(slot, ec=ec, filler=filler):
                    # slots 4-7: give the prior chunk's denominator DMAs
                    # time to land before the recip chain reads them
                    if ec > 0 and 4 <= slot < 8:
                        normalize_tb(ec - 1, slot - 4)
                    next(filler, None)

                attention_chunk(ec, drip)
                for _ in filler:   # drain remaining qk(ec+1) work
                    pass
        # xt pool released here (before the last attention chunk)

        with tc.tile_pool(name="wp", bufs=NEC) as wp_pool, \
             tc.tile_pool(name="stage", bufs=6) as stage_pool:
            wp_sb = []
            for ecn in range(NEC):
                t = wp_pool.tile([128, D], BF16, tag="wp", name=f"wpt{ecn}")
                nc.sync.dma_start(t[:], wp[ecn * 128:(ecn + 1) * 128, :])
                wp_sb.append(t)

            def proj_unit(ecs, out_t, tcn, ob, tag="qkps", act_copy=False):
                """One projection unit accumulating a subset of e-chunks
                into its own partial output (summed on the host)."""
                ps_p = ps_tile(tag, 2)
                for i, ecn in enumerate(ecs):
                    nc.tensor.matmul(
                        ps_p[:],
                        ut_sb[ecn][:, tcn * 128:(tcn + 1) * 128],
                        wp_sb[ecn][:, ob * 512:(ob + 1) * 512],
                        start=(i == 0), stop=(i == len(ecs) - 1))
                st = stage_pool.tile([128, 512], F32, tag="st", name="stg")
                if act_copy:
                    nc.scalar.activation(st[:], ps_p[:], COPY)
                else:
                    nc.vector.tensor_copy(st[:], ps_p[:])
                nc.sync.dma_start(
                    out_t[tcn * 128:(tcn + 1) * 128,
                          ob * 512:(ob + 1) * 512], st[:])

            # last attention chunk: one full projection pass (all 4 chunks),
            # i-block tb of chunk 3 unlocked (normalized) per flushed ib so
            # only the ib=3 slice remains for the tail.
            ECS = tuple(range(NEC))
            projU = {ib: [(tcn, ob) for tcn in range(4 * ib, 4 * ib + 4)
                          for ob in range(2)] for ib in range(NTB)}

            def drip3(slot):
                if 4 <= slot < 8:
                    normalize_tb(NEC - 2, slot - 4)
                elif slot >= 14:
                    ib = min((slot - 14) // 8, 2)
                    units = projU[ib]
                    # hold back 3 ungated ib2 units to cover the tail's
                    # denominator DMA -> recip -> broadcast latency
                    if units and (ib < 2 or len(units) > 3):
                        tcn, ob = units.pop(0)
                        proj_unit(ECS, outA, tcn, ob,
                                  act_copy=(slot % 2 == 1))

            def on_flush3(ib):
                if ib < NTB - 1:
                    normalize_tb(NEC - 1, ib)

            attention_chunk(NEC - 1, drip3, on_flush=on_flush3)
            # held-back ungated ib2 units keep the PE busy while the final
            # i-block's denominator DMA -> recip -> broadcast chain drains
            held = [u for ib in range(NTB - 1) for u in projU[ib]]
            for i, u in enumerate(held):
                proj_unit(ECS, outA, *u,
                          tag=("qkps" if i % 2 == 0 else "utps"),
                          act_copy=(i % 2 == 0))
            normalize_tb(NEC - 1, NTB - 1)
            for i, u in enumerate(projU[NTB - 1]):
                proj_unit(ECS, outA, *u,
                          tag=("qkps" if i % 2 == 0 else "utps"),
                          act_copy=(i % 2 == 1))



_NC_CACHE = None


def _get_nc():
    global _NC_CACHE
    if _NC_CACHE is None:
        _NC_CACHE = _build()
    return _NC_CACHE


def make_in_maps(embds, W_qkv, W_proj):
    embds = np.asarray(embds, dtype=np.float32)
    W_qkv = np.asarray(W_qkv, dtype=np.float32)
    W_proj = np.asarray(W_proj, dtype=np.float32)
    bf16 = ml_dtypes.bfloat16

    f8 = ml_dtypes.float8_e4m3

    mask_np = np.triu(np.ones((128, 128))).astype(bf16)
    emat_np = np.kron(np.eye(HL), np.ones((1, Dh))).astype(bf16)

    in_maps = []
    for c in range(NCORES):
        b, hg = c // 2, c % 2
        sl = slice(hg * DL, (hg + 1) * DL)
        xt = np.ascontiguousarray(embds[b].T)
        in_maps.append({
            "xT": xt.astype(bf16),
            "xT8": xt.astype(f8),
            "wk": np.ascontiguousarray(W_qkv[:, 0 * D:1 * D][:, sl]).astype(f8),
            "wq": np.ascontiguousarray(W_qkv[:, 1 * D:2 * D][:, sl]).astype(f8),
            "wv": np.ascontiguousarray(W_qkv[:, 2 * D:3 * D][:, sl]).astype(bf16),
            "wp": np.ascontiguousarray(W_proj[sl, :]).astype(bf16),
            "mask": mask_np,
            "emat": emat_np,
        })
    return in_maps


def gather_out(outs, b_proj):
    b_proj = np.asarray(b_proj, dtype=np.float32)
    full = np.empty((B, T, D), dtype=np.float32)
    for b in range(B):
        full[b] = outs[2 * b] + outs[2 * b + 1] + b_proj[None, :]
    return full


def kernel(embds, W_qkv, W_proj, b_proj):
    in_maps = make_in_maps(embds, W_qkv, W_proj)
    nc = _get_nc()
    res = run_bass_kernel_spmd(nc, in_maps, list(range(NCORES)))
    return gather_out([np.asarray(r["outA"], dtype=np.float32)
                       for r in res.results], b_proj)
